# revision 49
# baseline (speedup 1.0000x reference)
"""DFT spectrogram (nn_DftSpectrogram) Bass kernel for 8 Trainium2 NeuronCores.

Pure data parallel: 32 batch items -> 4 per core. Per item (T=96512 samples):
  - x is cast to fp16 on the host. Frame extraction AND transposition are
    fused into the input DMA: each 128-frame block is loaded with
    dma_start_transpose (HW XBAR, 16x128 tiles) straight from DRAM with the
    strided frame AP, landing in SBUF as [tap(chunk-major), frame] fp16 -
    exactly the stationary layout the PE needs. No on-chip transposes, no
    PSUM staging, no copies. All 20 block-transposes are issued up front and
    stream through the DGE while compute consumes them.
  - NO basis folding: per block, 4 accumulating matmuls contract the four
    128-tap chunks against a combined [cos|sin] basis ([128t, 512(k_r|k_i)])
    producing real|imag side by side in PSUM (fp16 x fp16 -> fp32).
  - squares: one wide Square per 2-block group over the PSUM tile, column-
    split between ACT and DVE to balance engine load; r^2+i^2 group-merged
    on GPSIMD (its only job); ln on ACT; bn_stats/bn_aggr on DVE.
  - blocks 0-3 of each item are normalized and shipped as soon as their
    stats are done; block 4 follows, keeping the kernel tail short.
  - normalized output is written fp16 and the host casts back to fp32 while
    fixing the layout to [k, frame].

1/(sqrt(var)+eps') is computed entirely on DVE (int bit-trick seed + two
Heron steps, 5e-7 rel; keeps ACT on one table set). eps compensation keeps
the algebra exact:
(fft-mean)/(std+1e-7) == (g-mean_g)/(std_g+2*ln(10)*1e-7).
"""
from contextlib import ExitStack

import numpy as np

import concourse.bass as bass
import concourse.tile as tile
from concourse import bacc, mybir
from concourse.bass_utils import run_bass_kernel_spmd

import os as _os

_NO_OUT = _os.environ.get("KERNEL_NO_OUT") == "1"  # timing experiments only
_NO_BACK = _os.environ.get("KERNEL_NO_BACK") == "1"
_NO_BN = _os.environ.get("KERNEL_NO_BN") == "1"

N_CORES = 8
B_FULL = 32
C_FULL = 1
T = 96512
NFFT = 512
KOUT = 256          # only lower half of the spectrum is kept
SHIFT = 160
F = (T - NFFT) // SHIFT + 1  # 601
BPC = B_FULL // N_CORES      # 4 items per core
EPS = 1e-7
CEPS = float(2.0 * np.log(10.0) * 1e-7)
F0S = (0, 128, 256, 384, 473)  # frame-block starts; last block overlaps by 39
FP32 = mybir.dt.float32
FP16 = mybir.dt.float16
MM_DT = FP16
SQ_DVE = 256  # columns of each group's Square taken by DVE (rest on ACT)


def _build(ctx: ExitStack, tc: "tile.TileContext", xh, wh, idh,
           outh, mm_dt, reps: int):
    nc = tc.nc
    AP = bass.AP
    AF = mybir.ActivationFunctionType

    consts = ctx.enter_context(tc.tile_pool(name="consts", bufs=1))
    ftpool = ctx.enter_context(tc.tile_pool(name="framesT", bufs=1))
    sqpool = ctx.enter_context(tc.tile_pool(name="sq", bufs=3))
    mpool = ctx.enter_context(tc.tile_pool(name="mag", bufs=2))
    glpool = ctx.enter_context(tc.tile_pool(name="gl", bufs=2))
    spool = ctx.enter_context(tc.tile_pool(name="stats", bufs=6))
    gnpool = ctx.enter_context(tc.tile_pool(name="gnorm", bufs=2))
    prip = ctx.enter_context(tc.tile_pool(name="pri", bufs=3, space="PSUM"))

    cs_sb = consts.tile([128, 4 * NFFT], mm_dt, tag="cs_sb")
    epsb = consts.tile([128, 1], FP32, tag="epsb")
    nc.vector.memset(epsb[:], EPS)
    # dummy Ln: makes the act-table pass load the natural_log set (which
    # also holds Square/Copy) ONCE during the DMA head instead of reloading
    # mid-stream at the first real Ln
    warm = consts.tile([1, 1], FP32, tag="warm")
    nc.scalar.activation(warm[:], epsb[0:1, :], mybir.ActivationFunctionType.Ln)

    def stage_mm(st, ftts):
        """DFT matmuls: per block, 4 accumulating [128c x 512(r|i)] mms."""
        fbs = st["fbs"]
        srcs = st.get("srcs") or [(ftts[fb], 0) for fb in fbs]
        pri = prip.tile([128, 2 * NFFT], FP32, tag="pri", name="pri")
        for loc, fb in enumerate(fbs):
            o = loc * NFFT
            ftt, base = srcs[loc]
            for c in range(4):
                nc.tensor.matmul(pri[:, o:o + NFFT],
                                 ftt[:, base + c * 128:base + (c + 1) * 128],
                                 cs_sb[:, c * NFFT:(c + 1) * NFFT],
                                 start=(c == 0), stop=(c == 3))
        st["pri"] = pri

    def stage_sq(st):
        """One wide Square over the group's PSUM tile (frees it for PE).
        Singleton groups square on DVE to shed ACT load."""
        nb = len(st["fbs"])
        pri = st["pri"]
        w = nb * NFFT
        sq = sqpool.tile([128, 2 * NFFT], FP32, tag="sq", name="sq")
        nc.scalar.activation(sq[:, 0:w], pri[:, 0:w], AF.Square)
        st["sq"] = sq

    def stage_magln(st):
        """Strided r^2+i^2 on GPSIMD, then Ln on ACT."""
        nb = len(st["fbs"])
        fb0 = st["fbs"][0]
        sq, gl = st["sq"], st["gl"]
        msum = mpool.tile([128, 2 * KOUT], FP32, tag="ms", name="ms")
        sqa = sq[:]
        nc.gpsimd.tensor_add(
            AP(msum[:].tensor, msum[:].offset,
               [list(msum[:].ap[0]), [KOUT, nb], [1, KOUT]]),
            AP(sqa.tensor, sqa.offset,
               [list(sqa.ap[0]), [NFFT, nb], [1, KOUT]]),
            AP(sqa.tensor, sqa.offset + KOUT,
               [list(sqa.ap[0]), [NFFT, nb], [1, KOUT]]))
        nc.scalar.activation(gl[:, fb0 * KOUT:(fb0 + nb) * KOUT],
                             msum[:, 0:nb * KOUT], AF.Ln, bias=epsb[:])

    def stage_bn(st):
        gl, mv3 = st["gl"], st["mv3"]
        for fb in st["fbs"]:
            bn6 = spool.tile([128, 6], FP32, tag="bn6", name="bn6")
            nc.vector.bn_stats(bn6[:], gl[:, fb * KOUT:(fb + 1) * KOUT])
            nc.vector.bn_aggr(mv3[:, fb, :], bn6[:])

    def stage_back(b, gl, mv, fb_lo=0, fb_hi=4, out_dmas=None):
        """rden = 1/(sqrt(var)+ceps) on DVE, then normalize; the output
        DMAs are appended to out_dmas for deferred issue (so they hit the
        SP ring only once their data is ready and never stall it)."""
        w = fb_hi - fb_lo + 1
        var = bass.AP(mv[:].tensor, mv[:].offset + 2 * fb_lo + 1,
                      [list(mv[:].ap[0]), [2, w]])
        sh = spool.tile([128, w], mybir.dt.int32, tag="sh", name="sh")
        nc.vector.tensor_scalar(sh[:], var.bitcast(mybir.dt.int32), 1, None,
                                op0=mybir.AluOpType.arith_shift_right)
        s0i = spool.tile([128, w], mybir.dt.int32, tag="s0i", name="s0i")
        nc.vector.tensor_scalar(s0i[:], sh[:], 0x1FBD1DF5, None,
                                op0=mybir.AluOpType.add)
        s_cur = s0i[:].bitcast(FP32)
        for it in range(1):
            hr = spool.tile([128, w], FP32, tag=f"hr{it}", name=f"hr{it}")
            nc.vector.reciprocal(hr[:], s_cur)
            ht = spool.tile([128, w], FP32, tag=f"ht{it}", name=f"ht{it}")
            nc.vector.tensor_mul(ht[:], var, hr[:])
            hs = spool.tile([128, w], FP32, tag=f"hs{it}", name=f"hs{it}")
            nc.vector.tensor_add(hs[:], s_cur, ht[:])
            hh = spool.tile([128, w], FP32, tag=f"hh{it}", name=f"hh{it}")
            nc.vector.tensor_scalar_mul(hh[:], hs[:], 0.5)
            s_cur = hh[:]
        uu = spool.tile([128, w], FP32, tag="uu", name="uu")
        nc.vector.tensor_scalar(uu[:], s_cur, 1.0, CEPS,
                                op0=mybir.AluOpType.mult,
                                op1=mybir.AluOpType.add)
        rden = spool.tile([128, w], FP32, tag="rden", name="rden")
        nc.vector.reciprocal(rden[:], uu[:])

        gn4 = None
        for fb in range(fb_lo, fb_hi + 1):
            gls = gl[:, fb * KOUT:(fb + 1) * KOUT]
            if fb < 4:
                if gn4 is None:
                    gn4 = gnpool.tile([128, 4 * KOUT], mm_dt, tag="gn4",
                                      name="gn4")
                gdst = gn4[:, fb * KOUT:(fb + 1) * KOUT]
            else:
                gdst = gnpool.tile([128, KOUT], mm_dt, tag="gn", name="gn")
            nc.vector.tensor_scalar(gdst, gls,
                                    mv[:, 2 * fb:2 * fb + 1],
                                    rden[:, fb - fb_lo:fb - fb_lo + 1],
                                    op0=mybir.AluOpType.subtract,
                                    op1=mybir.AluOpType.mult)
            if fb == 4:
                # frames 473..511 were already written by block 3
                out_dmas.append((outh.ap()[b, 512:601, :], gdst[39:128, :]))
        if gn4 is not None:
            nblk = min(fb_hi, 3) - fb_lo + 1
            dst = bass.AP(outh, b * F * KOUT + fb_lo * 128 * KOUT,
                          [[KOUT, 128], [128 * KOUT, nblk], [1, KOUT]])
            out_dmas.append(
                (dst, gn4[:, fb_lo * KOUT:(fb_lo + nblk) * KOUT].rearrange(
                    "p (f k) -> p f k", k=KOUT)))

    def body():
        # fused frame-extraction + transpose loads, all issued up front.
        # ftts[b][fb] is [tap(4-chunk-major), 128 frames] fp16.
        ftts = [[None] * 5 for _ in range(BPC)]

        def tr_dma(b, fb, eng):
            ftt = ftpool.tile([128, NFFT], mm_dt, tag=f"ftt{b}_{fb}",
                              name=f"ftt{b}_{fb}")
            src = AP(xh, b * T + SHIFT * F0S[fb],
                     [[SHIFT, 128], [1, NFFT]])
            eng.dma_start_transpose(
                ftt[:].rearrange("p (c f) -> p c f", f=128), src)
            ftts[b][fb] = ftt

        # all transposes on the SP ring (XBAR on the ACT ring corrupts
        # data on HW)
        tr_dma(0, 0, nc.sync)
        tr_dma(0, 1, nc.sync)
        if reps == 1:
            # basis in 4 chunk DMAs so the first matmul's chunk lands early
            for c in range(4):
                nc.sync.dma_start(cs_sb[:, c * NFFT:(c + 1) * NFFT],
                                  wh.ap()[c * 128:(c + 1) * 128, :])
        for b in range(BPC):
            for fb in range(5):
                if ftts[b][fb] is None:
                    tr_dma(b, fb, nc.sync)

        groups = []
        for b in range(BPC):
            gl = glpool.tile([128, 5 * KOUT], FP32, tag="gl", name="gl")
            mv = spool.tile([128, 10], FP32, tag="mv", name="mv")
            mv3 = mv[:].rearrange("p (f two) -> p f two", two=2)
            for fbs in ((0, 1), (2, 3), (4,)):
                groups.append({"b": b, "fbs": fbs, "gl": gl, "mv3": mv3,
                               "mv": mv})

        n = len(groups)
        pending_outs = []
        for i in range(n + 3):
            # flush output DMAs deferred from the previous step: their norm
            # data is ready by now, so they never dep-stall the SP ring
            for dst, srcv in pending_outs:
                if not _NO_OUT:
                    nc.sync.dma_start(dst, srcv)
            pending_outs = []
            if i < n:
                st = groups[i]
                stage_mm(st, ftts[st["b"]])
                stage_sq(st)
            # mag+ln lag one group so ln never blocks the next Square in
            # ACT's queue (pri recycling is paced by the Squares)
            if 1 <= i and i - 1 < n:
                stage_magln(groups[i - 1])
            # bn lags one more group, then flushes
            for j in ([i - 2] if i < n else range(i - 2, min(i, n))):
                if j < 0 or j >= n or groups[j].get("bn_done"):
                    continue
                st = groups[j]
                st["bn_done"] = True
                if not _NO_BN:
                    stage_bn(st)
                b = st["b"]
                if _NO_BACK or _NO_BN:
                    pass
                elif b < BPC - 1:
                    if st["fbs"][0] == 4:
                        stage_back(b, st["gl"], st["mv"], 0, 4,
                                   out_dmas=pending_outs)
                elif st["fbs"][0] == 2:
                    # last item: blocks 0-3 early so only block 4 sits in
                    # the kernel tail
                    stage_back(b, st["gl"], st["mv"], 0, 3,
                               out_dmas=pending_outs)
                elif st["fbs"][0] == 4:
                    stage_back(b, st["gl"], st["mv"], 4, 4,
                               out_dmas=pending_outs)

    if reps == 1:
        body()
    else:
        nc.sync.dma_start(cs_sb[:].rearrange("p (c k) -> p c k", k=NFFT),
                          wh.ap().rearrange("(c p) k -> p c k", p=128))
        with tc.For_i(0, reps, 1):
            body()


def build_nc(mm_dt=MM_DT, reps: int = 1):
    nc = bacc.Bacc("TRN2", target_bir_lowering=False, debug=False)
    xh = nc.dram_tensor("x", [BPC, T], mm_dt, kind="ExternalInput")
    wh = nc.dram_tensor("w", [4 * 128, 2 * KOUT], mm_dt, kind="ExternalInput")
    idh = nc.dram_tensor("ident", [128, 128], mm_dt, kind="ExternalInput")
    outh = nc.dram_tensor("out", [BPC, F, KOUT], mm_dt, kind="ExternalOutput")
    with tile.TileContext(nc) as tc, ExitStack() as ctx:
        _build(ctx, tc, xh, wh, idh, outh, mm_dt, reps)
    nc.compile()
    return nc


def make_in_maps(x, W_real, W_imag):
    xs = np.asarray(x, dtype=np.float32).reshape(B_FULL, T)
    Wr = np.asarray(W_real, np.float32)
    Wi = np.asarray(W_imag, np.float32)
    # combined [cos | sin] basis, taps on rows: w[t, 0:256] = Wr[k, t]^T,
    # w[t, 256:512] = Wi[k, t]^T
    w_dev = np.concatenate([Wr[:KOUT, :].T, Wi[:KOUT, :].T],
                           axis=1)  # [512, 512]
    hdt = np.float16
    xs16 = xs.astype(hdt)
    ident = np.eye(128, dtype=hdt)
    return [
        {"x": np.ascontiguousarray(xs16[i * BPC:(i + 1) * BPC]),
         "w": w_dev.astype(hdt), "ident": ident}
        for i in range(N_CORES)
    ]


_NC_CACHE = {}


def kernel(x, W_real, W_imag):
    key = (str(MM_DT), 1)
    if key not in _NC_CACHE:
        _NC_CACHE[key] = build_nc(MM_DT, 1)
    nc = _NC_CACHE[key]
    in_maps = make_in_maps(x, W_real, W_imag)
    res = run_bass_kernel_spmd(nc, in_maps, core_ids=list(range(N_CORES)))
    out = np.concatenate([np.asarray(r["out"]) for r in res.results], axis=0)
    out = out.astype(np.float32)
    out = np.ascontiguousarray(out.transpose(0, 2, 1))             # [32, K, F]
    return out.reshape(B_FULL, C_FULL, KOUT, F)
